# revision 21
# baseline (speedup 1.0000x reference)
"""Trainium2 Bass kernel for ContextEncoderModel (siamese LSTM encoder + MLP).

Reference computation (B=256, T=128, E=300, H=512, D=1024):
  lh = LSTM_left (left_embed,  left_lengths)  -> h at t=len-1   [B, H]
  rh = LSTM_right(right_embed, right_lengths) -> h at t=len-1   [B, H]
  x  = relu(concat([lh, rh]) @ trans_w)                          [B, D]
  x  = relu(x @ hidden_ws[0]); x = relu(x @ hidden_ws[1])        [B, D]

Sharding (8 cores): core i handles side s=i//4 (0=left, 1=right) and batch
shard q=i%4 (rows q*64:(q+1)*64).  The LSTM state is kept TRANSPOSED on
device: h^T/c^T live as [128 partitions(h-dim chunk), 4(chunk) x 64(batch)]
folded tiles, so the per-step recurrent matmul is
    z^T[gate-chunk m, b] += W[k-chunk, m-chunk].T @ h^T[k-chunk, b]
with the weights as the stationary operand and no per-step transposes.

PSUM layout: gates live in 2-bank PSUM group tiles shaped
[P, 2(gate), 4(h-chunk), 2*64(step,b)] -- tag A = (i, f) covering steps
[2g, 2g+2), tag B = (j, o) covering [2g+1, 2g+3), each double-buffered
(8 banks total).  All four h-chunks of a gate for one step form a single
strided AP, so each step needs only SIX activation instructions (sigmoid
f, sigmoid i, tanh j, sigmoid o, tanh c in two halves) and SIX vector ops
over [P, 256] tiles, instead of 16 + 20 per-chunk ops.  start=True is
issued exactly once per PSUM bank per accumulation group -- the hardware
clears has_written at bank granularity.

Scheduling: per step the PE runs the entire A block (i, f over all four
k-chunks) first, so the sigmoid chain starts ~1/3 into the step's PE work
and overlaps the B block; one group opens per step (phase-shifted A/B), so
a 24-matmul x-part burst (x_t @ W_x + bias via ones-row, FD=128) fills
the PE during every step's nonlinearity tail.  tanh(c) and h = tanh(c) *
sigmoid(o) are computed in two halves so next step's k0/k1 matmuls restart
the PE before the full h is done.

Final-state capture: the reference freezes state past t=len-1; running the
recurrence unmasked and latching h at t=len-1 is equivalent.  A host-built
one-hot mask [T, 128, 256] drives one copy_predicated per step into fh.

The MLP head needs concat(lh, rh):  trans_w is split into top/bottom 512
rows; each core computes its partial [D, 64] product and a pairwise
AllReduce (cores q and 4+q hold the same batch shard) produces the sum, then
both cores redundantly run the tiny 2-layer tail.  Host reads cores 0-3.
"""

import numpy as np
import ml_dtypes

import concourse.mybir as mybir
import concourse.tile as tile
from concourse import bacc
from concourse.bass_utils import run_bass_kernel_spmd

BF16 = mybir.dt.bfloat16
F32 = mybir.dt.float32
AF = mybir.ActivationFunctionType

B, T, E, H, D = 256, 128, 300, 512, 1024
NCORES = 8
BC = B // 4          # 64 batch rows per core (4 shards x 2 sides)
P = 128
G = 4 * H            # 2048 gate pre-activations
KH = H // P          # 4 h-dim chunks
KHB = KH * BC        # 256: folded free size of state tiles
KE_FULL = 2          # full 128-row chunks of E
E2 = E - KE_FULL * P  # 44 leftover rows
MG = G // P          # 16 gate-dim chunks
MD = D // P          # 8 D chunks
FORGET_BIAS = 1.0
MCH = 16             # mask chunk: steps per DMA
SG = 2               # steps per PSUM group
SGB = SG * BC        # 128


def _build(t_steps: int = T):
    nc = bacc.Bacc(
        "TRN2", target_bir_lowering=False, debug=False, num_devices=NCORES
    )
    TB = t_steps * BC
    mch = min(MCH, t_steps)

    def din(name, shape):
        return nc.dram_tensor(name, shape, BF16, kind="ExternalInput").ap()

    xt0 = din("xt0", [P, TB])
    xt1 = din("xt1", [P, TB])
    xt2 = din("xt2", [E2 + 1, TB])          # 44 embed rows + ones row
    wh = din("wh", [P, KH * G])             # wh[p, k*G+c] = W[300+k*128+p, c]
    wx01 = din("wx01", [P, 2 * G])
    wx2 = din("wx2", [E2 + 1, G])           # 44 rows + adjusted-bias row
    masks = nc.dram_tensor(
        "masks", [t_steps // mch, P, mch * KH * BC], mybir.dt.uint8,
        kind="ExternalInput",
    ).ap()
    twt = din("twt", [P, KH * D])           # side's 512 rows of trans_w
    hww = din("hww", [P, 2 * MD * D])       # both hidden layers
    out_t = nc.dram_tensor("out_t", [P, MD * BC], F32, kind="ExternalOutput").ap()

    with tile.TileContext(nc) as tc:
        with (
            tc.tile_pool(name="wpool", bufs=1) as wp,
            tc.tile_pool(name="state", bufs=1) as st,
            tc.tile_pool(name="gates", bufs=4) as gp,
            tc.tile_pool(name="mpool", bufs=2) as mp,
            tc.tile_pool(name="psum", bufs=1, space="PSUM") as psum,
            tc.tile_pool(name="dram", bufs=2, space="DRAM") as dp,
        ):
            # ---- resident weights / inputs ----
            XC = 32  # steps per xt DMA chunk
            nxc = max(1, t_steps // XC)
            xcs = min(XC, t_steps)
            wh_sb_k = [
                wp.tile([P, G], BF16, name=f"whk{k}") for k in range(KH)
            ]
            wx01_sb = wp.tile([P, 2 * G], BF16)
            wx2_sb = wp.tile([E2 + 1, G], BF16)
            twt_sb = wp.tile([P, KH * D], BF16)
            hww_sb = wp.tile([P, 2 * MD * D], BF16)
            xt0_c = []
            xt1_c = []
            xt2_c = []
            mchunk0 = mp.tile([P, mch * KH * BC], mybir.dt.uint8,
                             name="mchunk0", tag="mchunk")
            nc.sync.dma_start(mchunk0[:], masks[0, :, :])
            nc.sync.dma_start(wx01_sb[:], wx01[:])
            nc.sync.dma_start(wx2_sb[:], wx2[:])
            for ci in range(nxc):
                csl = slice(ci * xcs * BC, (ci + 1) * xcs * BC)
                x0 = wp.tile([P, xcs * BC], BF16, name=f"xt0c{ci}")
                x1 = wp.tile([P, xcs * BC], BF16, name=f"xt1c{ci}")
                x2 = wp.tile([E2 + 1, xcs * BC], BF16, name=f"xt2c{ci}")
                nc.sync.dma_start(x0[:], xt0[:, csl])
                if ci == 0:
                    for k in range(KH):
                        nc.sync.dma_start(wh_sb_k[k][:], wh[:, k * G:(k + 1) * G])
                nc.sync.dma_start(x1[:], xt1[:, csl])
                nc.sync.dma_start(x2[:], xt2[:, csl])
                xt0_c.append(x0)
                xt1_c.append(x1)
                xt2_c.append(x2)
            nc.sync.dma_start(twt_sb[:], twt[:])
            nc.sync.dma_start(hww_sb[:], hww[:])

            # ---- persistent state ----
            fh = st.tile([P, KHB], BF16)
            nc.vector.memset(fh[:], 0.0)
            h_cur = gp.tile([P, KHB], BF16, name="h0", tag="h")
            c_cur = gp.tile([P, KHB], F32, name="c0", tag="c")
            nc.vector.memset(h_cur[:], 0.0)
            nc.vector.memset(c_cur[:], 0.0)

            # ---- recurrence ----
            # Gate-chunk m for (which, gate-slot mi, h-chunk s):
            #   A: mi=0 -> i (m=s),    mi=1 -> f (m=8+s)
            #   B: mi=0 -> j (m=4+s),  mi=1 -> o (m=12+s)
            def gate_m(which, mi, s):
                return (s, 8 + s)[mi] if which == "A" else (4 + s, 12 + s)[mi]

            banks = {"A": {}, "B": {}}  # which -> {t0: [tile, t0, tlen, left]}

            pending_x = []  # deferred x-part matmuls, interleaved into
                            # the h-matmul stream to keep PE duty high

            def open_group(which, t0, tlen, defer=True):
                ps = psum.tile(
                    [P, 2, KH, SGB], F32, tag=f"zps{which}",
                    name=f"ps{which}_{t0}", bufs=2,
                )
                for mi in range(2):
                    for s in range(KH):
                        m = gate_m(which, mi, s)
                        msl = slice(m * P, (m + 1) * P)
                        m1sl = slice(G + m * P, G + (m + 1) * P)
                        # start=True must appear exactly once per PSUM BANK
                        # per accumulation group (it clears has_written at
                        # bank granularity): only the region at each 512-f32
                        # bank boundary leads.
                        bank_first = (s * SGB) % 512 == 0
                        done = 0
                        while done < tlen:
                            ci, o = divmod((t0 + done) * BC, xcs * BC)
                            seg = min(tlen - done, xcs - (t0 + done) % xcs)
                            rsl = slice(o, o + seg * BC)
                            osl = ps[:, mi, s, done * BC:(done + seg) * BC]
                            pending_x.append(
                                (osl, wx01_sb[:, msl], xt0_c[ci][:, rsl],
                                 bank_first and done == 0))
                            pending_x.append(
                                (osl, wx01_sb[:, m1sl], xt1_c[ci][:, rsl], False))
                            pending_x.append(
                                (osl, wx2_sb[:, msl], xt2_c[ci][:, rsl], False))
                            done += seg
                banks[which][t0] = [ps, t0, tlen, 2 * KH * KH * tlen]
                if not defer:
                    drain_x(len(pending_x))

            def drain_x(n):
                for _ in range(min(n, len(pending_x))):
                    osl, lw, rh, st = pending_x.pop(0)
                    nc.tensor.matmul(
                        osl, lhsT=lw, rhs=rh, start=st, stop=False,
                        skip_group_check=True,
                    )

            # A groups cover [0,2),[2,4),...; B groups [0,1),[1,3),[3,5),...
            # so exactly one group opens per step (x-burst smoothing), with
            # bufs=2 giving each tag double-buffered 2-bank slots.
            open_group("A", 0, min(SG, t_steps), defer=False)
            open_group("B", 0, 1, defer=False)
            if t_steps > 1:
                open_group("B", 1, min(SG, t_steps - 1), defer=False)

            def live_group(which, t):
                if which == "A":
                    return banks["A"][t - t % SG]
                return banks["B"][0 if t == 0 else (t if t % 2 == 1 else t - 1)]

            for t in range(t_steps):
                # h-part matmuls: the ENTIRE A block (i,f; all k) first, so
                # the sigmoid chain starts ~1/3 into the step's PE work and
                # runs concurrently with the B block and the x-burst.
                gA = live_group("A", t)
                gB = live_group("B", t)
                for (g, mi) in ((gA, 1), (gA, 0), (gB, 0), (gB, 1)):
                    ps, t0, tlen, left = g
                    ca = t - t0
                    for k in range(KH):
                        for s in range(KH):
                            m = gate_m("A" if g is gA else "B", mi, s)
                            left -= 1
                            nc.tensor.matmul(
                                ps[:, mi, s, ca * BC:(ca + 1) * BC],
                                lhsT=wh_sb_k[k][:, m * P:(m + 1) * P],
                                rhs=h_cur[:, k * BC:(k + 1) * BC],
                                start=False, stop=(left == 0),
                                skip_group_check=True,
                            )
                    g[3] = left
                    # interleave a slice of the pending x-burst: FD-128
                    # x-matmuls run near-100% PE duty, keeping the HAM
                    # clock gate warm through the low-duty h-stream.
                    # The queue MUST empty every step (chunk-crossing
                    # groups enqueue 48 instead of 24), else an x-matmul
                    # would issue after its group's h-matmuls.
                    if mi == 1 and g is gB:
                        drain_x(len(pending_x))
                    else:
                        drain_x(6)

                if t == 0:
                    mchunk = mchunk0
                elif t % MCH == 0 and t_steps >= MCH:
                    mchunk = mp.tile([P, MCH * KHB], mybir.dt.uint8, tag="mchunk")
                    nc.sync.dma_start(mchunk[:], masks[t // MCH, :, :])

                # gates: 6 activations + 6 vector ops for the whole step
                psA, tA = gA[0], gA[1]
                psB, tB = gB[0], gB[1]
                ca, cb = t - tA, t - tB
                sif = gp.tile([P, 2, KH, BC], F32, name="sif", tag="sif")
                tj = gp.tile([P, KHB], F32, name="tj", tag="tj")
                so = gp.tile([P, KHB], F32, name="so", tag="so")
                # f first: the c-path (ta = c*sf) unblocks earliest
                nc.scalar.activation(
                    sif[:, 1], psA[:, 1, :, ca * BC:(ca + 1) * BC], AF.Sigmoid
                )
                nc.scalar.activation(
                    sif[:, 0], psA[:, 0, :, ca * BC:(ca + 1) * BC], AF.Sigmoid
                )
                nc.scalar.activation(
                    tj[:], psB[:, 0, :, cb * BC:(cb + 1) * BC], AF.Tanh
                )
                nc.scalar.activation(
                    so[:], psB[:, 1, :, cb * BC:(cb + 1) * BC], AF.Sigmoid
                )
                ta = gp.tile([P, KHB], F32, name="ta", tag="ta")
                tb = gp.tile([P, KHB], F32, name="tb", tag="tb")
                cs = gp.tile([P, KHB], F32, name="cn", tag="c")
                tc_ = gp.tile([P, KHB], F32, name="tc", tag="tc")
                hs = gp.tile([P, KHB], BF16, name="hn", tag="h")
                nc.vector.tensor_mul(ta[:], c_cur[:], sif[:, 1, :, :])
                nc.vector.tensor_mul(tb[:], sif[:, 0, :, :], tj[:])
                nc.vector.tensor_add(cs[:], ta[:], tb[:])
                # split the tail in halves: h chunks 0-1 emerge early so
                # next step's k0/k1 matmuls restart the PE sooner
                HB = KHB // 2
                nc.scalar.activation(tc_[:, 0:HB], cs[:, 0:HB], AF.Tanh)
                nc.vector.tensor_mul(hs[:, 0:HB], tc_[:, 0:HB], so[:, 0:HB])
                nc.scalar.activation(tc_[:, HB:], cs[:, HB:], AF.Tanh)
                nc.vector.tensor_mul(hs[:, HB:], tc_[:, HB:], so[:, HB:])
                tt = t % MCH if t_steps >= MCH else t
                nc.vector.copy_predicated(
                    fh[:], mchunk[:, tt * KHB:(tt + 1) * KHB], hs[:]
                )
                h_cur = hs
                c_cur = cs

                # open the group two steps ahead: its x-part matmuls fill
                # the PE while this step's nonlinearity tail runs
                tn = t + 2
                if tn < t_steps:
                    if tn % 2 == 0:
                        open_group("A", tn, min(SG, t_steps - tn))
                    else:
                        open_group("B", tn, min(SG, t_steps - tn))

            # ---- MLP head ----
            # partial = (side rows of trans_w).T @ fh^T  -> [D, 64] transposed
            p_sb = st.tile([P, MD * BC], BF16)
            for m in range(MD):
                ps = psum.tile([P, 2, KH, SGB], F32, tag="zpsA", bufs=2,
                               name=f"hps{m}")
                for k in range(KH):
                    nc.tensor.matmul(
                        ps[:, 0, 0, 0:BC],
                        lhsT=twt_sb[:, k * D + m * P:k * D + (m + 1) * P],
                        rhs=fh[:, k * BC:(k + 1) * BC],
                        start=(k == 0), stop=(k == KH - 1),
                    )
                nc.vector.tensor_copy(p_sb[:, m * BC:(m + 1) * BC], ps[:, 0, 0, 0:BC])

            cin = dp.tile([P, MD * BC], BF16)
            cout = dp.tile([P, MD * BC], BF16)
            nc.sync.dma_start(cin[:], p_sb[:])
            nc.gpsimd.collective_compute(
                "AllReduce",
                mybir.AluOpType.add,
                replica_groups=[[0, 4], [1, 5], [2, 6], [3, 7]],
                ins=[cin.opt()],
                outs=[cout.opt()],
            )
            x1pre = st.tile([P, MD * BC], BF16)
            nc.sync.dma_start(x1pre[:], cout[:])
            xcur = st.tile([P, MD * BC], BF16)
            nc.scalar.activation(xcur[:], x1pre[:], AF.Relu)

            for layer in range(2):
                nxt = st.tile([P, MD * BC], BF16, tag=f"x{layer + 1}")
                out_f32 = None
                if layer == 1:
                    out_f32 = st.tile([P, MD * BC], F32, name="out_f32")
                for m in range(MD):
                    ps = psum.tile([P, 2, KH, SGB], F32, tag="zpsB", bufs=2,
                                   name=f"lps{layer}_{m}")
                    for k in range(MD):
                        off = (layer * MD + k) * D
                        nc.tensor.matmul(
                            ps[:, 0, 0, 0:BC],
                            lhsT=hww_sb[:, off + m * P:off + (m + 1) * P],
                            rhs=xcur[:, k * BC:(k + 1) * BC],
                            start=(k == 0), stop=(k == MD - 1),
                        )
                    if layer == 0:
                        nc.scalar.activation(
                            nxt[:, m * BC:(m + 1) * BC], ps[:, 0, 0, 0:BC], AF.Relu
                        )
                    else:
                        nc.scalar.activation(
                            out_f32[:, m * BC:(m + 1) * BC], ps[:, 0, 0, 0:BC],
                            AF.Relu
                        )
                xcur = nxt
            nc.sync.dma_start(out_t[:], out_f32[:])

    nc.compile()
    return nc


_BUILD_CACHE: dict = {}


def _get_nc(t_steps: int = T):
    if t_steps not in _BUILD_CACHE:
        _BUILD_CACHE[t_steps] = _build(t_steps)
    return _BUILD_CACHE[t_steps]


def _core_inputs(embed, lengths, Wf, bf, trans_w, hidden_ws, side, t_steps):
    """Build the per-core input dict. embed [BC,T,E] f32, lengths [BC] i32."""
    bf16 = ml_dtypes.bfloat16
    TB = t_steps * BC

    # x transposed: xt[e, t, b]
    xt = np.ascontiguousarray(
        embed[:, :t_steps, :].transpose(2, 1, 0)
    ).astype(bf16)  # [E, t_steps, BC]
    xt0 = xt[0:P].reshape(P, TB)
    xt1 = xt[P:2 * P].reshape(P, TB)
    xt2 = np.empty((E2 + 1, TB), dtype=bf16)
    xt2[:E2] = xt[2 * P:E].reshape(E2, TB)
    xt2[E2] = np.ones(TB, dtype=bf16)

    Wb = Wf.astype(np.float32)
    wh = np.ascontiguousarray(
        Wb[E:].reshape(KH, P, G).transpose(1, 0, 2).reshape(P, KH * G)
    ).astype(bf16)
    wx01 = np.ascontiguousarray(
        Wb[0:2 * P].reshape(2, P, G).transpose(1, 0, 2).reshape(P, 2 * G)
    ).astype(bf16)
    badj = bf.astype(np.float32).copy()
    badj[2 * H:3 * H] += FORGET_BIAS
    wx2 = np.empty((E2 + 1, G), dtype=bf16)
    wx2[:E2] = Wb[2 * P:E].astype(bf16)
    wx2[E2] = badj.astype(bf16)

    # one-hot capture masks, replicated across partitions and h-chunks
    m_tb = np.zeros((t_steps, BC), dtype=np.uint8)
    cap = np.minimum(lengths.astype(np.int64), t_steps) - 1
    m_tb[cap, np.arange(BC)] = 1
    mch = min(MCH, t_steps)
    masks = np.ascontiguousarray(
        np.broadcast_to(
            m_tb[:, None, None, None, :], (t_steps, P, 1, KH, BC)
        ).reshape(t_steps // mch, mch, P, KH * BC).transpose(0, 2, 1, 3)
    ).reshape(t_steps // mch, P, mch * KH * BC)

    tw = trans_w[side * H:(side + 1) * H].astype(np.float32)
    twt = np.ascontiguousarray(
        tw.reshape(KH, P, D).transpose(1, 0, 2).reshape(P, KH * D)
    ).astype(bf16)
    hww = np.ascontiguousarray(
        hidden_ws.astype(np.float32).reshape(2, MD, P, D)
        .transpose(2, 0, 1, 3).reshape(P, 2 * MD * D)
    ).astype(bf16)

    return dict(xt0=xt0, xt1=xt1, xt2=xt2, wh=wh, wx01=wx01, wx2=wx2,
                masks=masks, twt=twt, hww=hww)


def prepare_in_maps(left_embed, right_embed, left_lengths, right_lengths,
                    W_left, b_left, W_right, b_right, trans_w, hidden_ws,
                    t_steps=T):
    in_maps = []
    for core in range(NCORES):
        side, q = divmod(core, 4)
        rows = slice(q * BC, (q + 1) * BC)
        if side == 0:
            emb, ln, Wf, bf = left_embed[rows], left_lengths[rows], W_left, b_left
        else:
            emb, ln, Wf, bf = right_embed[rows], right_lengths[rows], W_right, b_right
        in_maps.append(
            _core_inputs(np.asarray(emb), np.asarray(ln), np.asarray(Wf),
                         np.asarray(bf), np.asarray(trans_w),
                         np.asarray(hidden_ws), side, t_steps)
        )
    return in_maps


def _assemble(results):
    out = np.empty((B, D), dtype=np.float32)
    for q in range(4):
        # out_t [P, MD, BC] with out_t[p, m, b] = y[q*64+b, m*128+p]
        ot = results[q]["out_t"].reshape(P, MD, BC)
        out[q * BC:(q + 1) * BC] = ot.transpose(2, 1, 0).reshape(BC, D)
    return out


def kernel(left_embed, right_embed, left_lengths, right_lengths,
           W_left, b_left, W_right, b_right, trans_w, hidden_ws):
    nc = _get_nc(T)
    in_maps = prepare_in_maps(
        left_embed, right_embed, left_lengths, right_lengths,
        W_left, b_left, W_right, b_right, trans_w, hidden_ws, T
    )
    r = run_bass_kernel_spmd(nc, in_maps, list(range(NCORES)))
    return _assemble(r.results)


def run_traced(inputs, t_steps=T, **trace_kwargs):
    """test.py helper: run with NTFF tracing, return (output, BassKernelResults)."""
    nc = _get_nc(t_steps)
    in_maps = prepare_in_maps(t_steps=t_steps, **inputs)
    r = run_bass_kernel_spmd(
        nc, in_maps, list(range(NCORES)), trace=True, **trace_kwargs
    )
    return _assemble(r.results), r
